# revision 42
# baseline (speedup 1.0000x reference)
"""Multi-head attention forward on 8 Trainium2 NeuronCores.

Sharding: core = (batch b in 0..2, head-group hg in 0..4); each core owns
4 of the 16 heads for one batch element. Q/K/V projections are computed
per-core for its 256 head-dims; attention runs per head with scores kept
transposed (S^T[k, q]) so no on-chip transposes are needed; the output
projection is row-sharded over W_o, producing a per-core partial Y that
the host sums over the 4 head-groups of each batch.

All matmul operands are fp16 (PSUM accumulation stays fp32). V is stored
in head-pair blocks [V_even | ones | junk | V_odd] (192 cols); the PV
stationary is the 128-wide window starting at offset 0 (even head: ctx
rows 0:64, denom row 64) or offset 64 (odd head: denom row 0, ctx rows
64:128), so each head's softmax denominator comes free.

Single fully-pipelined phase: the attention loop runs j-outer/h-inner in
score groups of 2 k-tiles; PV matmuls are emitted from a deferred queue
(gated on V-projection availability), and V-proj / Q-proj / output-proj
matmuls are pumped as filler into the PE stream so the tensor engine
never waits on the (near-critical) scalar-engine exp chain. The scalar
engine runs ONLY exp; all PSUM->SBUF copies go to vector/gpsimd.
"""

import sys

for _p in ("/opt/trn_rl_repo", "/opt/pypackages"):
    if _p not in sys.path:
        sys.path.append(_p)

from collections import deque
from contextlib import ExitStack

import numpy as np

import concourse.bass as bass
import concourse.tile as tile
from concourse import bacc, mybir
from concourse import bass_utils

P = 128
B = 2
S = 2048          # sequence length
D = 1024          # model dim
H = 16            # total heads
DK = 64           # head dim
HL = 4            # heads per core
CL = HL * DK      # local head dims per core (256)
NJ = 4            # 512-wide q-slices
NS = 512
NI = D // P       # 8 contraction tiles over model dim
NK = S // P       # 16 key tiles
NQB = S // P      # 16 query blocks for the output projection
VPB = 192         # V pair block: V_even(64) | ones(1) | junk(63) | V_odd(64)
VPAD = 2 * VPB    # 384 cols for 2 head pairs
NG = NK // 2      # score groups of 2 k-tiles per (h, j)

F32 = mybir.dt.float32
F16 = mybir.dt.float16
EXP = mybir.ActivationFunctionType.Exp


def build_nc():
    nc = bacc.Bacc("TRN2", target_bir_lowering=False, debug=False)

    xqT = nc.dram_tensor("xqT", [D, S], F16, kind="ExternalInput")
    xkT = nc.dram_tensor("xkT", [D, S], F16, kind="ExternalInput")
    xvT = nc.dram_tensor("xvT", [D, S], F16, kind="ExternalInput")
    wqT = nc.dram_tensor("wqT", [D, CL], F16, kind="ExternalInput")
    wkT = nc.dram_tensor("wkT", [D, CL], F16, kind="ExternalInput")
    wvT = nc.dram_tensor("wvT", [D, CL], F16, kind="ExternalInput")
    woT = nc.dram_tensor("woT", [CL, D], F16, kind="ExternalInput")
    y = nc.dram_tensor("y", [S, D], F16, kind="ExternalOutput")

    with tile.TileContext(nc) as tc, ExitStack() as ctx:
        wpool = ctx.enter_context(tc.tile_pool(name="w", bufs=1))
        big = ctx.enter_context(tc.tile_pool(name="big", bufs=1))
        xpool = ctx.enter_context(tc.tile_pool(name="xs", bufs=48))
        epool = ctx.enter_context(tc.tile_pool(name="ex", bufs=16))
        spool = ctx.enter_context(tc.tile_pool(name="sm", bufs=4))
        ypool = ctx.enter_context(tc.tile_pool(name="yo", bufs=3))
        psS = ctx.enter_context(tc.tile_pool(name="psS", bufs=3, space="PSUM"))
        psB = ctx.enter_context(tc.tile_pool(name="psB", bufs=2, space="PSUM"))

        # Resident weights
        wq_sb = wpool.tile([P, NI, CL], F16)
        wk_sb = wpool.tile([P, NI, CL], F16)
        wv_sb = wpool.tile([P, NI, CL], F16)
        wo_sb = wpool.tile([P, CL // P, D], F16)

        # Resident activations: Q^T / K^T with head dims on partitions
        # ([128, ot, s]); V in head-pair blocks; normalized ctx^T.
        qT_sb = big.tile([P, 2, S], F16)
        kT_sb = big.tile([P, 2, S], F16)
        v_sb = big.tile([P, NK, VPAD], F16)
        cT_sb = big.tile([P, 2, S], F16)

        nc.gpsimd.memset(v_sb[:], 1.0)

        # ---- DMA emission, priority ordered ---------------------------
        # Weight loads are per-i-tile contiguous row blocks (fast), not one
        # big strided gather. Priority: wk, xk j0, xq j0, wq, wv, xk j1-3,
        # xv j0-1, xq j1, xv j2-3, xq j2-3, wo.
        xk_t = {}
        xq_t = {}
        xv_t = {}
        ndma = [0]

        def dma_eng():
            ndma[0] += 1
            return nc.sync if ndma[0] % 2 else nc.gpsimd

        def load_w(w_sb, w_dram):
            for i in range(NI):
                dma_eng().dma_start(
                    w_sb[:, i, :], w_dram.ap()[i * P:(i + 1) * P, :])

        def load_x(x_t, x_dram, j, nm):
            for i in range(NI):
                t = xpool.tile([P, NS], F16, tag="x", name=nm)
                dma_eng().dma_start(
                    t[:], x_dram.ap()[i * P:(i + 1) * P, j * NS:(j + 1) * NS])
                x_t[(i, j)] = t

        load_w(wk_sb, wkT)
        load_x(xk_t, xkT, 0, "xk_t")
        load_w(wq_sb, wqT)
        load_x(xq_t, xqT, 0, "xq_t")
        load_w(wv_sb, wvT)
        for j in range(1, NJ):
            load_x(xk_t, xkT, j, "xk_t")
        load_x(xv_t, xvT, 0, "xv_t")
        load_x(xv_t, xvT, 1, "xv_t")
        load_x(xq_t, xqT, 1, "xq_t")
        load_x(xv_t, xvT, 2, "xv_t")
        load_x(xv_t, xvT, 3, "xv_t")
        load_x(xq_t, xqT, 2, "xq_t")
        load_x(xq_t, xqT, 3, "xq_t")
        for ct in range(2):
            dma_eng().dma_start(
                wo_sb[:, ct, :], woT.ap()[ct * P:(ct + 1) * P, :])

        def proj_half(x_t, w_sb, out_sb, j, ot, pool, tag, eng):
            # out_sb[:, ot, j*NS:...] = (X @ W.T)^T for this core's dims
            ps = pool.tile([P, NS], F32, tag=tag, name="ps")
            for i in range(NI):
                nc.tensor.matmul(
                    ps[:],
                    w_sb[:, i, ot * P:(ot + 1) * P],
                    x_t[(i, j)][:],
                    start=(i == 0),
                    stop=(i == NI - 1),
                )
            cp = getattr(eng, "tensor_copy", None) or eng.copy
            cp(out_sb[:, ot, j * NS:(j + 1) * NS], ps[:])

        # ---- Prefix: K and Q slice-0 projections, ot=0 half only ------
        # (heads h0/h1 read only the ot=0 half; ot=1 is deferred to filler
        # since h2/h3 don't run until step 16 of the slice)
        proj_half(xk_t, wk_sb, kT_sb, 0, 0, psS, "sc", nc.vector)
        proj_half(xq_t, wq_sb, qT_sb, 0, 0, psS, "sc", nc.vector)

        # ---- Filler closures ------------------------------------------
        # filler entries are (ready_fn, go_fn): pumped only once ready, so a
        # closure whose inputs trail in program order can't block the stream.
        filler = deque()
        norms_done = [0] * NJ   # norms emitted per slice
        v_avail = [0]   # number of v k-tiles whose projection is emitted

        def pump():
            if filler and filler[0][0]():
                filler.popleft()[1]()
                return True
            return False

        def k_chunk(j, ot):
            def go():
                proj_half(xk_t, wk_sb, kT_sb, j, ot, psS, "sc", nc.vector)
            return go

        def v_chunk(t):
            # V projection for key tile t (128 keys), packed into pair blocks
            def go():
                ps = psS.tile([P, CL], F32, tag="sc", name="psv")
                jv, c0 = t // 4, (t % 4) * P
                for i in range(NI):
                    nc.tensor.matmul(
                        ps[:],
                        xv_t[(i, jv)][:, c0:c0 + P],
                        wv_sb[:, i, :],
                        start=(i == 0),
                        stop=(i == NI - 1),
                    )
                vv = v_sb[:, t].rearrange("p (pr c) -> p pr c", c=VPB)
                pv = ps[:].rearrange("p (pr hc) -> p pr hc", hc=2 * DK)
                nc.vector.tensor_copy(vv[:, :, 0:DK], pv[:, :, 0:DK])
                nc.vector.tensor_copy(vv[:, :, 2 * DK:3 * DK], pv[:, :, DK:2 * DK])
                v_avail[0] = t + 1
            return go

        def q0_chunk():
            def go():
                proj_half(xq_t, wq_sb, qT_sb, 0, 1, psS, "sc", nc.scalar)
            return go

        early = deque()
        for j in range(1, NJ):
            early.append(k_chunk(j, 0))
        early.extend(v_chunk(t) for t in range(8))
        early.append(k_chunk(0, 1))
        early.append(q0_chunk())
        early.extend(v_chunk(t) for t in range(8, 12))
        for j in range(1, NJ):
            early.append(k_chunk(j, 1))
        early.extend(v_chunk(t) for t in range(12, NK))

        def q_chunk(j, ot):
            # Q projection for slice j, half ot; two closures (4 mm each)
            ref = {}

            def go1():
                ps = psS.tile([P, NS], F32, tag="sc", name="psq")
                ref["ps"] = ps
                for i in range(4):
                    nc.tensor.matmul(
                        ps[:], wq_sb[:, i, ot * P:(ot + 1) * P],
                        xq_t[(i, j)][:], start=(i == 0), stop=False)

            def go2():
                ps = ref["ps"]
                for i in range(4, NI):
                    nc.tensor.matmul(
                        ps[:], wq_sb[:, i, ot * P:(ot + 1) * P],
                        xq_t[(i, j)][:], start=False, stop=(i == NI - 1))
                # scalar copy: vector's in-order queue runs deep behind norm
                # chains, which would delay the next slice's first scores
                nc.scalar.copy(qT_sb[:, ot, j * NS:(j + 1) * NS], ps[:])
            return [go1, go2]

        always = lambda: True
        for ot in range(2):
            filler.extend((always, c) for c in q_chunk(1, ot))

        def o_chunk(qb):
            # output projection for query block qb; two closures, each
            # matmul+copy so the borrowed scores-pool slot frees quickly
            ref = {}

            def go1():
                yp = psS.tile([P, NS], F32, tag="sc", name="yp0")
                for ct in range(2):
                    nc.tensor.matmul(
                        yp[:], cT_sb[:, ct, qb * P:(qb + 1) * P],
                        wo_sb[:, ct, 0:NS], start=(ct == 0), stop=(ct == 1))
                ysb = ypool.tile([P, D], F16, tag="y", name="ysb")
                ref[0] = ysb
                nc.vector.tensor_copy(ysb[:, 0:NS], yp[:])

            def go2():
                yp = psS.tile([P, NS], F32, tag="sc", name="yp1")
                for ct in range(2):
                    nc.tensor.matmul(
                        yp[:], cT_sb[:, ct, qb * P:(qb + 1) * P],
                        wo_sb[:, ct, NS:D], start=(ct == 0), stop=(ct == 1))
                ysb = ref[0]
                nc.vector.tensor_copy(ysb[:, NS:D], yp[:])
                nc.sync.dma_start(y.ap()[qb * P:(qb + 1) * P, :], ysb[:])
            return [go1, go2]

        # ---- Attention: j-outer, h-inner, score groups of 2 -----------
        pvq = deque()   # deferred PV emissions: (ex, k0, k1, state)

        def emit_norm(st):
            h, j, ctx_ps = st["h"], st["j"], st["ctx"]
            pr0 = (h % 2) * 64
            ot = h // 2
            drow = 64 * (1 - h % 2)
            norms_done[j] += 1
            rec = spool.tile([1, NS], F32, tag="rec", name="rec")
            if drow == 0:
                # odd heads: denominator already on partition 0 — skip the
                # partition-shifting copy (custom DVE ops cannot shift)
                nc.vector.reciprocal_approx_fast(rec[:], ctx_ps[0:1, :])
            else:
                den = spool.tile([1, NS], F32, tag="den", name="den")
                nc.vector.tensor_copy(den[:], ctx_ps[drow:drow + 1, :])
                nc.vector.reciprocal_approx_fast(rec[:], den[:])
            bc = spool.tile([P, NS], F32, tag="bc", name="bc")
            nc.gpsimd.partition_broadcast(bc[:], rec[:])
            nc.vector.tensor_mul(
                cT_sb[pr0:pr0 + 64, ot, j * NS:(j + 1) * NS],
                ctx_ps[pr0:pr0 + 64, :],
                bc[pr0:pr0 + 64, :],
            )

        def drain_pv(max_n, min_keep=1):
            n = 0
            while len(pvq) > min_keep and n < max_n:
                ex, k0, k1, st = pvq[0]
                if k1 >= v_avail[0]:
                    break
                pvq.popleft()
                h = st["h"]
                vcol = (h // 2) * VPB + (h % 2) * DK
                nc.tensor.matmul(
                    st["ctx"][:], v_sb[:, k0, vcol:vcol + P], ex[:, 0],
                    start=(k0 == 0), stop=False)
                nc.tensor.matmul(
                    st["ctx"][:], v_sb[:, k1, vcol:vcol + P], ex[:, 1],
                    start=False, stop=(k1 == NK - 1))
                st["emitted"] += 1
                if st["emitted"] == NG:
                    emit_norm(st)
                n += 1

        for j in range(NJ):
            if 1 <= j < NJ - 1:
                # queue the next slice's Q projection ahead of older filler
                for ot in (1, 0):
                    c2, c1 = q_chunk(j + 1, ot)[::-1]
                    filler.appendleft((always, c2))
                    filler.appendleft((always, c1))
            cool = [3 if j >= 1 else 0]
            for h in range(HL):
                pr0 = (h % 2) * 64
                ot = h // 2
                ctx_ps = psB.tile([P, NS], F32, tag="ctx", name="ctx")
                st = {"h": h, "j": j, "ctx": ctx_ps, "emitted": 0}
                for g in range(NG):
                    sp = psS.tile([P, 2, NS], F32, tag="sc", name="sp")
                    for idx in range(2):
                        k = 2 * g + idx
                        nc.tensor.matmul(
                            sp[:, idx],
                            kT_sb[pr0:pr0 + 64, ot, k * P:(k + 1) * P],
                            qT_sb[pr0:pr0 + 64, ot, j * NS:(j + 1) * NS],
                            start=True,
                            stop=True,
                        )
                    ex = epool.tile([P, 2, NS], F16, tag="ex", name="ex")
                    nc.scalar.activation(ex[:], sp[:], EXP, scale=0.125)
                    pvq.append((ex, 2 * g, 2 * g + 1, st))
                    drain_pv(2, min_keep=2)
                    # pump filler at the end of the step: the scores matmul
                    # issues first, so the exp stream is never delayed
                    if early:
                        # pace the early K/V chunks to the DMA arrival rate:
                        # a chunk pumped ahead of its data blocks the stream
                        early.popleft()()
                        if early and (len(early) > 16 or len(early) <= 4):
                            early.popleft()()
                    elif cool[0]:
                        # at a slice start, let the exp pipeline refill its
                        # run-ahead before filler competes for PSUM slots
                        cool[0] -= 1
                    else:
                        pump()
            # slice done: queue its output projection (gated on its norms)
            while early:
                early.popleft()()
            ready = (lambda jj: lambda: norms_done[jj] == HL)(j)
            for qb in range(4 * j, 4 * j + 4):
                filler.extend((ready, c) for c in o_chunk(qb))

        # tail: finish all PVs, then all remaining filler
        drain_pv(len(pvq), min_keep=0)
        while filler:
            filler.popleft()[1]()

    nc.compile()
    return nc


_NC = None


def _get_nc():
    global _NC
    if _NC is None:
        _NC = build_nc()
    return _NC


def _shard_inputs(Query, Key, Value, W_q, W_k, W_v, W_o):
    in_maps = []
    xT = {}
    for b in range(B):
        xT[b] = (
            np.ascontiguousarray(Query[b].T).astype(np.float16),
            np.ascontiguousarray(Key[b].T).astype(np.float16),
            np.ascontiguousarray(Value[b].T).astype(np.float16),
        )
    for b in range(B):
        for hg in range(4):
            r0 = hg * CL
            in_maps.append({
                "xqT": xT[b][0],
                "xkT": xT[b][1],
                "xvT": xT[b][2],
                "wqT": np.ascontiguousarray(W_q[r0:r0 + CL, :].T).astype(np.float16),
                "wkT": np.ascontiguousarray(W_k[r0:r0 + CL, :].T).astype(np.float16),
                "wvT": np.ascontiguousarray(W_v[r0:r0 + CL, :].T).astype(np.float16),
                "woT": np.ascontiguousarray(W_o[:, r0:r0 + CL].T).astype(np.float16),
            })
    return in_maps


def _reference_np(Query, Key, Value, mask, W_q, W_k, W_v, W_o):
    # Fallback for a non-trivial mask (never hit for the spec'd inputs).
    out = np.empty((B, S, D), dtype=np.float32)
    m = np.broadcast_to(mask, (1, 1, S, S))[0, 0]
    for b in range(B):
        Q = (Query[b] @ W_q.T).reshape(S, H, DK).transpose(1, 0, 2)
        K = (Key[b] @ W_k.T).reshape(S, H, DK).transpose(1, 0, 2)
        V = (Value[b] @ W_v.T).reshape(S, H, DK).transpose(1, 0, 2)
        ctx = np.empty((H, S, DK), dtype=np.float32)
        for h in range(H):
            s = (Q[h] @ K[h].T) / np.sqrt(DK)
            s = np.where(m == 0, -1e9, s)
            s -= s.max(axis=-1, keepdims=True)
            e = np.exp(s)
            ctx[h] = (e / e.sum(axis=-1, keepdims=True)) @ V[h]
        out[b] = ctx.transpose(1, 0, 2).reshape(S, D) @ W_o.T
    return out


def kernel(Query, Key, Value, mask, W_q, W_k, W_v, W_o, **_ignored):
    Query = np.asarray(Query, dtype=np.float32)
    Key = np.asarray(Key, dtype=np.float32)
    Value = np.asarray(Value, dtype=np.float32)
    W_q = np.asarray(W_q, dtype=np.float32)
    W_k = np.asarray(W_k, dtype=np.float32)
    W_v = np.asarray(W_v, dtype=np.float32)
    W_o = np.asarray(W_o, dtype=np.float32)

    if not np.all(np.asarray(mask) != 0):
        return _reference_np(Query, Key, Value, np.asarray(mask),
                             W_q, W_k, W_v, W_o)

    nc = _get_nc()
    in_maps = _shard_inputs(Query, Key, Value, W_q, W_k, W_v, W_o)
    res = bass_utils.run_bass_kernel_spmd(nc, in_maps, core_ids=list(range(8)))
    out = np.zeros((B, S, D), dtype=np.float32)
    for b in range(B):
        for hg in range(4):
            out[b] += res.results[b * 4 + hg]["y"].astype(np.float32)
    return out


# revision 43
# speedup vs baseline: 1.2451x; 1.2451x over previous
"""Multi-head attention forward on 8 Trainium2 NeuronCores.

Sharding: core = (batch b in 0..2, head-group hg in 0..4); each core owns
4 of the 16 heads for one batch element. Q/K/V projections are computed
per-core for its 256 head-dims; attention runs per head with scores kept
transposed (S^T[k, q]) so no on-chip transposes are needed; the output
projection is row-sharded over W_o, producing a per-core partial Y that
the host sums over the 4 head-groups of each batch.

All matmul operands are fp16 (PSUM accumulation stays fp32). V is stored
in head-pair blocks [V_even | ones | junk | V_odd] (192 cols); the PV
stationary is the 128-wide window starting at offset 0 (even head: ctx
rows 0:64, denom row 64) or offset 64 (odd head: denom row 0, ctx rows
64:128), so each head's softmax denominator comes free.

Single fully-pipelined phase: the attention loop runs j-outer/h-inner in
score groups of 2 k-tiles; PV matmuls are emitted from a deferred queue
(gated on V-projection availability), and V-proj / Q-proj / output-proj
matmuls are pumped as filler into the PE stream so the tensor engine
never waits on the (near-critical) scalar-engine exp chain. The scalar
engine runs ONLY exp; all PSUM->SBUF copies go to vector/gpsimd.
"""

import sys

for _p in ("/opt/trn_rl_repo", "/opt/pypackages"):
    if _p not in sys.path:
        sys.path.append(_p)

from collections import deque
from contextlib import ExitStack

import numpy as np

import concourse.bass as bass
import concourse.tile as tile
from concourse import bacc, mybir
from concourse import bass_utils

P = 128
B = 2
S = 2048          # sequence length
D = 1024          # model dim
H = 16            # total heads
DK = 64           # head dim
HL = 4            # heads per core
CL = HL * DK      # local head dims per core (256)
NJ = 4            # 512-wide q-slices
NS = 512
NI = D // P       # 8 contraction tiles over model dim
NK = S // P       # 16 key tiles
NQB = S // P      # 16 query blocks for the output projection
VPB = 192         # V pair block: V_even(64) | ones(1) | junk(63) | V_odd(64)
VPAD = 2 * VPB    # 384 cols for 2 head pairs
NG = NK // 2      # score groups of 2 k-tiles per (h, j)

F32 = mybir.dt.float32
F16 = mybir.dt.float16
EXP = mybir.ActivationFunctionType.Exp


def build_nc():
    nc = bacc.Bacc("TRN2", target_bir_lowering=False, debug=False)

    xqT = nc.dram_tensor("xqT", [D, S], F16, kind="ExternalInput")
    xkT = nc.dram_tensor("xkT", [D, S], F16, kind="ExternalInput")
    xvT = nc.dram_tensor("xvT", [D, S], F16, kind="ExternalInput")
    wqT = nc.dram_tensor("wqT", [D, CL], F16, kind="ExternalInput")
    wkT = nc.dram_tensor("wkT", [D, CL], F16, kind="ExternalInput")
    wvT = nc.dram_tensor("wvT", [D, CL], F16, kind="ExternalInput")
    woT = nc.dram_tensor("woT", [CL, D], F16, kind="ExternalInput")
    y = nc.dram_tensor("y", [S, D], F16, kind="ExternalOutput")

    with tile.TileContext(nc) as tc, ExitStack() as ctx:
        wpool = ctx.enter_context(tc.tile_pool(name="w", bufs=1))
        big = ctx.enter_context(tc.tile_pool(name="big", bufs=1))
        xpool = ctx.enter_context(tc.tile_pool(name="xs", bufs=48))
        epool = ctx.enter_context(tc.tile_pool(name="ex", bufs=16))
        spool = ctx.enter_context(tc.tile_pool(name="sm", bufs=4))
        ypool = ctx.enter_context(tc.tile_pool(name="yo", bufs=3))
        psS = ctx.enter_context(tc.tile_pool(name="psS", bufs=3, space="PSUM"))
        psB = ctx.enter_context(tc.tile_pool(name="psB", bufs=2, space="PSUM"))

        # Resident weights
        wq_sb = wpool.tile([P, NI, CL], F16)
        wk_sb = wpool.tile([P, NI, CL], F16)
        wv_sb = wpool.tile([P, NI, CL], F16)
        wo_sb = wpool.tile([P, CL // P, D], F16)

        # Resident activations: Q^T / K^T with head dims on partitions
        # ([128, ot, s]); V in head-pair blocks; normalized ctx^T.
        qT_sb = big.tile([P, 2, S], F16)
        kT_sb = big.tile([P, 2, S], F16)
        v_sb = big.tile([P, NK, VPAD], F16)
        cT_sb = big.tile([P, 2, S], F16)

        nc.gpsimd.memset(v_sb[:], 1.0)

        # ---- DMA emission, priority ordered ---------------------------
        # Weight loads are per-i-tile contiguous row blocks (fast), not one
        # big strided gather. Priority: wk, xk j0, xq j0, wq, wv, xk j1-3,
        # xv j0-1, xq j1, xv j2-3, xq j2-3, wo.
        xk_t = {}
        xq_t = {}
        xv_t = {}
        ndma = [0]

        def dma_eng():
            ndma[0] += 1
            return nc.sync if ndma[0] % 2 else nc.gpsimd

        def load_w(w_sb, w_dram):
            for i in range(NI):
                dma_eng().dma_start(
                    w_sb[:, i, :], w_dram.ap()[i * P:(i + 1) * P, :])

        def load_x(x_t, x_dram, j, nm):
            for i in range(NI):
                t = xpool.tile([P, NS], F16, tag="x", name=nm)
                dma_eng().dma_start(
                    t[:], x_dram.ap()[i * P:(i + 1) * P, j * NS:(j + 1) * NS])
                x_t[(i, j)] = t

        load_w(wk_sb, wkT)
        load_x(xk_t, xkT, 0, "xk_t")
        load_w(wq_sb, wqT)
        load_x(xq_t, xqT, 0, "xq_t")
        load_w(wv_sb, wvT)
        for j in range(1, NJ):
            load_x(xk_t, xkT, j, "xk_t")
        load_x(xv_t, xvT, 0, "xv_t")
        load_x(xv_t, xvT, 1, "xv_t")
        load_x(xq_t, xqT, 1, "xq_t")
        load_x(xv_t, xvT, 2, "xv_t")
        load_x(xv_t, xvT, 3, "xv_t")
        load_x(xq_t, xqT, 2, "xq_t")
        load_x(xq_t, xqT, 3, "xq_t")
        for ct in range(2):
            dma_eng().dma_start(
                wo_sb[:, ct, :], woT.ap()[ct * P:(ct + 1) * P, :])

        def proj_half(x_t, w_sb, out_sb, j, ot, pool, tag, eng):
            # out_sb[:, ot, j*NS:...] = (X @ W.T)^T for this core's dims
            ps = pool.tile([P, NS], F32, tag=tag, name="ps")
            for i in range(NI):
                nc.tensor.matmul(
                    ps[:],
                    w_sb[:, i, ot * P:(ot + 1) * P],
                    x_t[(i, j)][:],
                    start=(i == 0),
                    stop=(i == NI - 1),
                )
            cp = getattr(eng, "tensor_copy", None) or eng.copy
            cp(out_sb[:, ot, j * NS:(j + 1) * NS], ps[:])

        # ---- Prefix: K and Q projections for slice 0 only -------------
        # (borrows the scores-pool PSUM slots, which are idle pre-attention)
        for ot in range(2):
            proj_half(xk_t, wk_sb, kT_sb, 0, ot, psS, "sc", nc.vector)
        for ot in range(2):
            proj_half(xq_t, wq_sb, qT_sb, 0, ot, psS, "sc", nc.vector)

        # ---- Filler closures ------------------------------------------
        # filler entries are (ready_fn, go_fn): pumped only once ready, so a
        # closure whose inputs trail in program order can't block the stream.
        filler = deque()
        norms_done = [0] * NJ   # norms emitted per slice
        v_avail = [0]   # number of v k-tiles whose projection is emitted

        def pump():
            if filler and filler[0][0]():
                filler.popleft()[1]()
                return True
            return False

        def k_chunk(j, ot):
            def go():
                proj_half(xk_t, wk_sb, kT_sb, j, ot, psS, "sc", nc.vector)
            return go

        def v_chunk(t):
            # V projection for key tile t (128 keys), packed into pair blocks
            def go():
                ps = psS.tile([P, CL], F32, tag="sc", name="psv")
                jv, c0 = t // 4, (t % 4) * P
                for i in range(NI):
                    nc.tensor.matmul(
                        ps[:],
                        xv_t[(i, jv)][:, c0:c0 + P],
                        wv_sb[:, i, :],
                        start=(i == 0),
                        stop=(i == NI - 1),
                    )
                vv = v_sb[:, t].rearrange("p (pr c) -> p pr c", c=VPB)
                pv = ps[:].rearrange("p (pr hc) -> p pr hc", hc=2 * DK)
                nc.vector.tensor_copy(vv[:, :, 0:DK], pv[:, :, 0:DK])
                nc.vector.tensor_copy(vv[:, :, 2 * DK:3 * DK], pv[:, :, DK:2 * DK])
                v_avail[0] = t + 1
            return go

        early = deque()
        for j in range(1, NJ):
            for ot in range(2):
                early.append(k_chunk(j, ot))
        early.extend(v_chunk(t) for t in range(NK))

        def q_chunk(j, ot):
            # Q projection for slice j, half ot; two closures (4 mm each)
            ref = {}

            def go1():
                ps = psS.tile([P, NS], F32, tag="sc", name="psq")
                ref["ps"] = ps
                for i in range(4):
                    nc.tensor.matmul(
                        ps[:], wq_sb[:, i, ot * P:(ot + 1) * P],
                        xq_t[(i, j)][:], start=(i == 0), stop=False)

            def go2():
                ps = ref["ps"]
                for i in range(4, NI):
                    nc.tensor.matmul(
                        ps[:], wq_sb[:, i, ot * P:(ot + 1) * P],
                        xq_t[(i, j)][:], start=False, stop=(i == NI - 1))
                # scalar copy: vector's in-order queue runs deep behind norm
                # chains, which would delay the next slice's first scores
                nc.scalar.copy(qT_sb[:, ot, j * NS:(j + 1) * NS], ps[:])
            return [go1, go2]

        always = lambda: True
        for ot in range(2):
            filler.extend((always, c) for c in q_chunk(1, ot))

        def o_chunk(qb):
            # output projection for query block qb; two closures, each
            # matmul+copy so the borrowed scores-pool slot frees quickly
            ref = {}

            def go1():
                yp = psS.tile([P, NS], F32, tag="sc", name="yp0")
                for ct in range(2):
                    nc.tensor.matmul(
                        yp[:], cT_sb[:, ct, qb * P:(qb + 1) * P],
                        wo_sb[:, ct, 0:NS], start=(ct == 0), stop=(ct == 1))
                ysb = ypool.tile([P, D], F16, tag="y", name="ysb")
                ref[0] = ysb
                nc.vector.tensor_copy(ysb[:, 0:NS], yp[:])

            def go2():
                yp = psS.tile([P, NS], F32, tag="sc", name="yp1")
                for ct in range(2):
                    nc.tensor.matmul(
                        yp[:], cT_sb[:, ct, qb * P:(qb + 1) * P],
                        wo_sb[:, ct, NS:D], start=(ct == 0), stop=(ct == 1))
                ysb = ref[0]
                nc.vector.tensor_copy(ysb[:, NS:D], yp[:])
                nc.sync.dma_start(y.ap()[qb * P:(qb + 1) * P, :], ysb[:])
            return [go1, go2]

        # ---- Attention: j-outer, h-inner, score groups of 2 -----------
        pvq = deque()   # deferred PV emissions: (ex, k0, k1, state)

        def emit_norm(st):
            h, j, ctx_ps = st["h"], st["j"], st["ctx"]
            pr0 = (h % 2) * 64
            ot = h // 2
            drow = 64 * (1 - h % 2)
            norms_done[j] += 1
            rec = spool.tile([1, NS], F32, tag="rec", name="rec")
            if drow == 0:
                # odd heads: denominator already on partition 0 — skip the
                # partition-shifting copy (custom DVE ops cannot shift)
                nc.vector.reciprocal_approx_fast(rec[:], ctx_ps[0:1, :])
            else:
                den = spool.tile([1, NS], F32, tag="den", name="den")
                nc.vector.tensor_copy(den[:], ctx_ps[drow:drow + 1, :])
                nc.vector.reciprocal_approx_fast(rec[:], den[:])
            bc = spool.tile([P, NS], F32, tag="bc", name="bc")
            nc.gpsimd.partition_broadcast(bc[:], rec[:])
            nc.vector.tensor_mul(
                cT_sb[pr0:pr0 + 64, ot, j * NS:(j + 1) * NS],
                ctx_ps[pr0:pr0 + 64, :],
                bc[pr0:pr0 + 64, :],
            )

        def drain_pv(max_n, min_keep=1):
            n = 0
            while len(pvq) > min_keep and n < max_n:
                ex, k0, k1, st = pvq[0]
                if k1 >= v_avail[0]:
                    break
                pvq.popleft()
                h = st["h"]
                vcol = (h // 2) * VPB + (h % 2) * DK
                nc.tensor.matmul(
                    st["ctx"][:], v_sb[:, k0, vcol:vcol + P], ex[:, 0],
                    start=(k0 == 0), stop=False)
                nc.tensor.matmul(
                    st["ctx"][:], v_sb[:, k1, vcol:vcol + P], ex[:, 1],
                    start=False, stop=(k1 == NK - 1))
                st["emitted"] += 1
                if st["emitted"] == NG:
                    emit_norm(st)
                n += 1

        for j in range(NJ):
            if 1 <= j < NJ - 1:
                # queue the next slice's Q projection ahead of older filler
                for ot in (1, 0):
                    c2, c1 = q_chunk(j + 1, ot)[::-1]
                    filler.appendleft((always, c2))
                    filler.appendleft((always, c1))
            cool = [3 if j >= 1 else 0]
            for h in range(HL):
                pr0 = (h % 2) * 64
                ot = h // 2
                ctx_ps = psB.tile([P, NS], F32, tag="ctx", name="ctx")
                st = {"h": h, "j": j, "ctx": ctx_ps, "emitted": 0}
                for g in range(NG):
                    sp = psS.tile([P, 2, NS], F32, tag="sc", name="sp")
                    for idx in range(2):
                        k = 2 * g + idx
                        nc.tensor.matmul(
                            sp[:, idx],
                            kT_sb[pr0:pr0 + 64, ot, k * P:(k + 1) * P],
                            qT_sb[pr0:pr0 + 64, ot, j * NS:(j + 1) * NS],
                            start=True,
                            stop=True,
                        )
                    ex = epool.tile([P, 2, NS], F16, tag="ex", name="ex")
                    nc.scalar.activation(ex[:], sp[:], EXP, scale=0.125)
                    pvq.append((ex, 2 * g, 2 * g + 1, st))
                    drain_pv(2, min_keep=2)
                    # pump filler at the end of the step: the scores matmul
                    # issues first, so the exp stream is never delayed
                    if early:
                        # pace the early K/V chunks to the DMA arrival rate:
                        # a chunk pumped ahead of its data blocks the stream
                        early.popleft()()
                        if early and (len(early) > 16 or len(early) <= 4):
                            early.popleft()()
                    elif cool[0]:
                        # at a slice start, let the exp pipeline refill its
                        # run-ahead before filler competes for PSUM slots
                        cool[0] -= 1
                    else:
                        pump()
            # slice done: queue its output projection (gated on its norms)
            while early:
                early.popleft()()
            ready = (lambda jj: lambda: norms_done[jj] == HL)(j)
            for qb in range(4 * j, 4 * j + 4):
                filler.extend((ready, c) for c in o_chunk(qb))

        # tail: finish all PVs, then all remaining filler
        drain_pv(len(pvq), min_keep=0)
        while filler:
            filler.popleft()[1]()

    nc.compile()
    return nc


_NC = None


def _get_nc():
    global _NC
    if _NC is None:
        _NC = build_nc()
    return _NC


def _shard_inputs(Query, Key, Value, W_q, W_k, W_v, W_o):
    in_maps = []
    xT = {}
    for b in range(B):
        xT[b] = (
            np.ascontiguousarray(Query[b].T).astype(np.float16),
            np.ascontiguousarray(Key[b].T).astype(np.float16),
            np.ascontiguousarray(Value[b].T).astype(np.float16),
        )
    for b in range(B):
        for hg in range(4):
            r0 = hg * CL
            in_maps.append({
                "xqT": xT[b][0],
                "xkT": xT[b][1],
                "xvT": xT[b][2],
                "wqT": np.ascontiguousarray(W_q[r0:r0 + CL, :].T).astype(np.float16),
                "wkT": np.ascontiguousarray(W_k[r0:r0 + CL, :].T).astype(np.float16),
                "wvT": np.ascontiguousarray(W_v[r0:r0 + CL, :].T).astype(np.float16),
                "woT": np.ascontiguousarray(W_o[:, r0:r0 + CL].T).astype(np.float16),
            })
    return in_maps


def _reference_np(Query, Key, Value, mask, W_q, W_k, W_v, W_o):
    # Fallback for a non-trivial mask (never hit for the spec'd inputs).
    out = np.empty((B, S, D), dtype=np.float32)
    m = np.broadcast_to(mask, (1, 1, S, S))[0, 0]
    for b in range(B):
        Q = (Query[b] @ W_q.T).reshape(S, H, DK).transpose(1, 0, 2)
        K = (Key[b] @ W_k.T).reshape(S, H, DK).transpose(1, 0, 2)
        V = (Value[b] @ W_v.T).reshape(S, H, DK).transpose(1, 0, 2)
        ctx = np.empty((H, S, DK), dtype=np.float32)
        for h in range(H):
            s = (Q[h] @ K[h].T) / np.sqrt(DK)
            s = np.where(m == 0, -1e9, s)
            s -= s.max(axis=-1, keepdims=True)
            e = np.exp(s)
            ctx[h] = (e / e.sum(axis=-1, keepdims=True)) @ V[h]
        out[b] = ctx.transpose(1, 0, 2).reshape(S, D) @ W_o.T
    return out


def kernel(Query, Key, Value, mask, W_q, W_k, W_v, W_o, **_ignored):
    Query = np.asarray(Query, dtype=np.float32)
    Key = np.asarray(Key, dtype=np.float32)
    Value = np.asarray(Value, dtype=np.float32)
    W_q = np.asarray(W_q, dtype=np.float32)
    W_k = np.asarray(W_k, dtype=np.float32)
    W_v = np.asarray(W_v, dtype=np.float32)
    W_o = np.asarray(W_o, dtype=np.float32)

    if not np.all(np.asarray(mask) != 0):
        return _reference_np(Query, Key, Value, np.asarray(mask),
                             W_q, W_k, W_v, W_o)

    nc = _get_nc()
    in_maps = _shard_inputs(Query, Key, Value, W_q, W_k, W_v, W_o)
    res = bass_utils.run_bass_kernel_spmd(nc, in_maps, core_ids=list(range(8)))
    out = np.zeros((B, S, D), dtype=np.float32)
    for b in range(B):
        for hg in range(4):
            out[b] += res.results[b * 4 + hg]["y"].astype(np.float32)
    return out


# revision 44
# speedup vs baseline: 1.2485x; 1.0027x over previous
"""Multi-head attention forward on 8 Trainium2 NeuronCores.

Sharding: core = (batch b in 0..2, head-group hg in 0..4); each core owns
4 of the 16 heads for one batch element. Q/K/V projections are computed
per-core for its 256 head-dims; attention runs per head with scores kept
transposed (S^T[k, q]) so no on-chip transposes are needed; the output
projection is row-sharded over W_o, producing a per-core partial Y that
the host sums over the 4 head-groups of each batch.

All matmul operands are fp16 (PSUM accumulation stays fp32). V is stored
in head-pair blocks [V_even | ones | junk | V_odd] (192 cols); the PV
stationary is the 128-wide window starting at offset 0 (even head: ctx
rows 0:64, denom row 64) or offset 64 (odd head: denom row 0, ctx rows
64:128), so each head's softmax denominator comes free.

Single fully-pipelined phase: the attention loop runs j-outer/h-inner in
score groups of 2 k-tiles; PV matmuls are emitted from a deferred queue
(gated on V-projection availability), and V-proj / Q-proj / output-proj
matmuls are pumped as filler into the PE stream so the tensor engine
never waits on the (near-critical) scalar-engine exp chain. The scalar
engine runs ONLY exp; all PSUM->SBUF copies go to vector/gpsimd.
"""

import sys

for _p in ("/opt/trn_rl_repo", "/opt/pypackages"):
    if _p not in sys.path:
        sys.path.append(_p)

from collections import deque
from contextlib import ExitStack

import numpy as np

import concourse.bass as bass
import concourse.tile as tile
from concourse import bacc, mybir
from concourse import bass_utils

P = 128
B = 2
S = 2048          # sequence length
D = 1024          # model dim
H = 16            # total heads
DK = 64           # head dim
HL = 4            # heads per core
CL = HL * DK      # local head dims per core (256)
NJ = 4            # 512-wide q-slices
NS = 512
NI = D // P       # 8 contraction tiles over model dim
NK = S // P       # 16 key tiles
NQB = S // P      # 16 query blocks for the output projection
VPB = 192         # V pair block: V_even(64) | ones(1) | junk(63) | V_odd(64)
VPAD = 2 * VPB    # 384 cols for 2 head pairs
NG = NK // 2      # score groups of 2 k-tiles per (h, j)

F32 = mybir.dt.float32
F16 = mybir.dt.float16
EXP = mybir.ActivationFunctionType.Exp


def build_nc():
    nc = bacc.Bacc("TRN2", target_bir_lowering=False, debug=False)

    xqT = nc.dram_tensor("xqT", [D, S], F16, kind="ExternalInput")
    xkT = nc.dram_tensor("xkT", [D, S], F16, kind="ExternalInput")
    xvT = nc.dram_tensor("xvT", [D, S], F16, kind="ExternalInput")
    wqT = nc.dram_tensor("wqT", [D, CL], F16, kind="ExternalInput")
    wkT = nc.dram_tensor("wkT", [D, CL], F16, kind="ExternalInput")
    wvT = nc.dram_tensor("wvT", [D, CL], F16, kind="ExternalInput")
    woT = nc.dram_tensor("woT", [CL, D], F16, kind="ExternalInput")
    y = nc.dram_tensor("y", [S, D], F16, kind="ExternalOutput")

    with tile.TileContext(nc) as tc, ExitStack() as ctx:
        wpool = ctx.enter_context(tc.tile_pool(name="w", bufs=1))
        big = ctx.enter_context(tc.tile_pool(name="big", bufs=1))
        xpool = ctx.enter_context(tc.tile_pool(name="xs", bufs=48))
        epool = ctx.enter_context(tc.tile_pool(name="ex", bufs=16))
        spool = ctx.enter_context(tc.tile_pool(name="sm", bufs=4))
        ypool = ctx.enter_context(tc.tile_pool(name="yo", bufs=3))
        psS = ctx.enter_context(tc.tile_pool(name="psS", bufs=3, space="PSUM"))
        psB = ctx.enter_context(tc.tile_pool(name="psB", bufs=2, space="PSUM"))

        # Resident weights
        wq_sb = wpool.tile([P, NI, CL], F16)
        wk_sb = wpool.tile([P, NI, CL], F16)
        wv_sb = wpool.tile([P, NI, CL], F16)
        wo_sb = wpool.tile([P, CL // P, D], F16)

        # Resident activations: Q^T / K^T with head dims on partitions
        # ([128, ot, s]); V in head-pair blocks; normalized ctx^T.
        qT_sb = big.tile([P, 2, S], F16)
        kT_sb = big.tile([P, 2, S], F16)
        v_sb = big.tile([P, NK, VPAD], F16)
        cT_sb = big.tile([P, 2, S], F16)

        nc.gpsimd.memset(v_sb[:], 1.0)

        # ---- DMA emission, priority ordered ---------------------------
        # Weight loads are per-i-tile contiguous row blocks (fast), not one
        # big strided gather. Priority: wk, xk j0, xq j0, wq, wv, xk j1-3,
        # xv j0-1, xq j1, xv j2-3, xq j2-3, wo.
        xk_t = {}
        xq_t = {}
        xv_t = {}
        ndma = [0]

        def dma_eng():
            ndma[0] += 1
            return nc.sync if ndma[0] % 2 else nc.gpsimd

        def load_w(w_sb, w_dram):
            for i in range(NI):
                dma_eng().dma_start(
                    w_sb[:, i, :], w_dram.ap()[i * P:(i + 1) * P, :])

        def load_x(x_t, x_dram, j, nm):
            for i in range(NI):
                t = xpool.tile([P, NS], F16, tag="x", name=nm)
                dma_eng().dma_start(
                    t[:], x_dram.ap()[i * P:(i + 1) * P, j * NS:(j + 1) * NS])
                x_t[(i, j)] = t

        load_w(wk_sb, wkT)
        load_x(xk_t, xkT, 0, "xk_t")
        load_w(wq_sb, wqT)
        load_x(xq_t, xqT, 0, "xq_t")
        load_w(wv_sb, wvT)
        for j in range(1, NJ):
            load_x(xk_t, xkT, j, "xk_t")
        load_x(xv_t, xvT, 0, "xv_t")
        load_x(xv_t, xvT, 1, "xv_t")
        load_x(xq_t, xqT, 1, "xq_t")
        load_x(xv_t, xvT, 2, "xv_t")
        load_x(xv_t, xvT, 3, "xv_t")
        load_x(xq_t, xqT, 2, "xq_t")
        load_x(xq_t, xqT, 3, "xq_t")
        for ct in range(2):
            dma_eng().dma_start(
                wo_sb[:, ct, :], woT.ap()[ct * P:(ct + 1) * P, :])

        def proj_half(x_t, w_sb, out_sb, j, ot, pool, tag, eng):
            # out_sb[:, ot, j*NS:...] = (X @ W.T)^T for this core's dims
            ps = pool.tile([P, NS], F32, tag=tag, name="ps")
            for i in range(NI):
                nc.tensor.matmul(
                    ps[:],
                    w_sb[:, i, ot * P:(ot + 1) * P],
                    x_t[(i, j)][:],
                    start=(i == 0),
                    stop=(i == NI - 1),
                )
            cp = getattr(eng, "tensor_copy", None) or eng.copy
            cp(out_sb[:, ot, j * NS:(j + 1) * NS], ps[:])

        # ---- Prefix: K and Q projections for slice 0 only -------------
        # (borrows the scores-pool PSUM slots, which are idle pre-attention)
        for ot in range(2):
            proj_half(xk_t, wk_sb, kT_sb, 0, ot, psS, "sc", nc.vector)
        for ot in range(2):
            proj_half(xq_t, wq_sb, qT_sb, 0, ot, psS, "sc", nc.vector)

        # ---- Filler closures ------------------------------------------
        # filler entries are (ready_fn, go_fn): pumped only once ready, so a
        # closure whose inputs trail in program order can't block the stream.
        filler = deque()
        norms_done = [0] * NJ   # norms emitted per slice
        v_avail = [0]   # number of v k-tiles whose projection is emitted

        def pump():
            if filler and filler[0][0]():
                filler.popleft()[1]()
                return True
            return False

        def k_chunk(j, ot):
            def go():
                proj_half(xk_t, wk_sb, kT_sb, j, ot, psS, "sc", nc.vector)
            return go

        def v_chunk(t):
            # V projection for key tile t (128 keys), packed into pair blocks
            def go():
                ps = psS.tile([P, CL], F32, tag="sc", name="psv")
                jv, c0 = t // 4, (t % 4) * P
                for i in range(NI):
                    nc.tensor.matmul(
                        ps[:],
                        xv_t[(i, jv)][:, c0:c0 + P],
                        wv_sb[:, i, :],
                        start=(i == 0),
                        stop=(i == NI - 1),
                    )
                vv = v_sb[:, t].rearrange("p (pr c) -> p pr c", c=VPB)
                pv = ps[:].rearrange("p (pr hc) -> p pr hc", hc=2 * DK)
                nc.vector.tensor_copy(vv[:, :, 0:DK], pv[:, :, 0:DK])
                nc.vector.tensor_copy(vv[:, :, 2 * DK:3 * DK], pv[:, :, DK:2 * DK])
                v_avail[0] = t + 1
            return go

        early = deque()
        for j in range(1, NJ):
            for ot in range(2):
                early.append(k_chunk(j, ot))
        early.extend(v_chunk(t) for t in range(NK))

        def q_chunk(j, ot):
            # Q projection for slice j, half ot; two closures (4 mm each)
            ref = {}

            def go1():
                ps = psS.tile([P, NS], F32, tag="sc", name="psq")
                ref["ps"] = ps
                for i in range(4):
                    nc.tensor.matmul(
                        ps[:], wq_sb[:, i, ot * P:(ot + 1) * P],
                        xq_t[(i, j)][:], start=(i == 0), stop=False)

            def go2():
                ps = ref["ps"]
                for i in range(4, NI):
                    nc.tensor.matmul(
                        ps[:], wq_sb[:, i, ot * P:(ot + 1) * P],
                        xq_t[(i, j)][:], start=False, stop=(i == NI - 1))
                # ot=0 feeds the next slice's first scores: scalar copy, as
                # vector's in-order queue runs deep behind norm chains.
                # ot=1 isn't read until h2 (step 16), so vector is fine and
                # the exp stream isn't displaced twice.
                if ot == 0:
                    nc.scalar.copy(qT_sb[:, ot, j * NS:(j + 1) * NS], ps[:])
                else:
                    nc.vector.tensor_copy(
                        qT_sb[:, ot, j * NS:(j + 1) * NS], ps[:])
            return [go1, go2]

        always = lambda: True
        for ot in range(2):
            filler.extend((always, c) for c in q_chunk(1, ot))

        def o_chunk(qb):
            # output projection for query block qb; two closures, each
            # matmul+copy so the borrowed scores-pool slot frees quickly
            ref = {}

            def go1():
                yp = psS.tile([P, NS], F32, tag="sc", name="yp0")
                for ct in range(2):
                    nc.tensor.matmul(
                        yp[:], cT_sb[:, ct, qb * P:(qb + 1) * P],
                        wo_sb[:, ct, 0:NS], start=(ct == 0), stop=(ct == 1))
                ysb = ypool.tile([P, D], F16, tag="y", name="ysb")
                ref[0] = ysb
                nc.vector.tensor_copy(ysb[:, 0:NS], yp[:])

            def go2():
                yp = psS.tile([P, NS], F32, tag="sc", name="yp1")
                for ct in range(2):
                    nc.tensor.matmul(
                        yp[:], cT_sb[:, ct, qb * P:(qb + 1) * P],
                        wo_sb[:, ct, NS:D], start=(ct == 0), stop=(ct == 1))
                ysb = ref[0]
                nc.vector.tensor_copy(ysb[:, NS:D], yp[:])
                nc.sync.dma_start(y.ap()[qb * P:(qb + 1) * P, :], ysb[:])
            return [go1, go2]

        # ---- Attention: j-outer, h-inner, score groups of 2 -----------
        pvq = deque()   # deferred PV emissions: (ex, k0, k1, state)

        def emit_norm(st):
            h, j, ctx_ps = st["h"], st["j"], st["ctx"]
            pr0 = (h % 2) * 64
            ot = h // 2
            drow = 64 * (1 - h % 2)
            norms_done[j] += 1
            rec = spool.tile([1, NS], F32, tag="rec", name="rec")
            if drow == 0:
                # odd heads: denominator already on partition 0 — skip the
                # partition-shifting copy (custom DVE ops cannot shift)
                nc.vector.reciprocal_approx_fast(rec[:], ctx_ps[0:1, :])
            else:
                den = spool.tile([1, NS], F32, tag="den", name="den")
                nc.vector.tensor_copy(den[:], ctx_ps[drow:drow + 1, :])
                nc.vector.reciprocal_approx_fast(rec[:], den[:])
            bc = spool.tile([P, NS], F32, tag="bc", name="bc")
            nc.gpsimd.partition_broadcast(bc[:], rec[:])
            nc.vector.tensor_mul(
                cT_sb[pr0:pr0 + 64, ot, j * NS:(j + 1) * NS],
                ctx_ps[pr0:pr0 + 64, :],
                bc[pr0:pr0 + 64, :],
            )

        def drain_pv(max_n, min_keep=1):
            n = 0
            while len(pvq) > min_keep and n < max_n:
                ex, k0, k1, st = pvq[0]
                if k1 >= v_avail[0]:
                    break
                pvq.popleft()
                h = st["h"]
                vcol = (h // 2) * VPB + (h % 2) * DK
                nc.tensor.matmul(
                    st["ctx"][:], v_sb[:, k0, vcol:vcol + P], ex[:, 0],
                    start=(k0 == 0), stop=False)
                nc.tensor.matmul(
                    st["ctx"][:], v_sb[:, k1, vcol:vcol + P], ex[:, 1],
                    start=False, stop=(k1 == NK - 1))
                st["emitted"] += 1
                if st["emitted"] == NG:
                    emit_norm(st)
                n += 1

        for j in range(NJ):
            if 1 <= j < NJ - 1:
                # queue the next slice's Q projection ahead of older filler
                for ot in (1, 0):
                    c2, c1 = q_chunk(j + 1, ot)[::-1]
                    filler.appendleft((always, c2))
                    filler.appendleft((always, c1))
            cool = [3 if j >= 1 else 0]
            for h in range(HL):
                pr0 = (h % 2) * 64
                ot = h // 2
                ctx_ps = psB.tile([P, NS], F32, tag="ctx", name="ctx")
                st = {"h": h, "j": j, "ctx": ctx_ps, "emitted": 0}
                for g in range(NG):
                    sp = psS.tile([P, 2, NS], F32, tag="sc", name="sp")
                    for idx in range(2):
                        k = 2 * g + idx
                        nc.tensor.matmul(
                            sp[:, idx],
                            kT_sb[pr0:pr0 + 64, ot, k * P:(k + 1) * P],
                            qT_sb[pr0:pr0 + 64, ot, j * NS:(j + 1) * NS],
                            start=True,
                            stop=True,
                        )
                    ex = epool.tile([P, 2, NS], F16, tag="ex", name="ex")
                    nc.scalar.activation(ex[:], sp[:], EXP, scale=0.125)
                    pvq.append((ex, 2 * g, 2 * g + 1, st))
                    drain_pv(2, min_keep=2)
                    # pump filler at the end of the step: the scores matmul
                    # issues first, so the exp stream is never delayed
                    if early:
                        # pace the early K/V chunks to the DMA arrival rate:
                        # a chunk pumped ahead of its data blocks the stream
                        early.popleft()()
                        if early and (len(early) > 16 or len(early) <= 4):
                            early.popleft()()
                    elif cool[0]:
                        # at a slice start, let the exp pipeline refill its
                        # run-ahead before filler competes for PSUM slots
                        cool[0] -= 1
                    else:
                        pump()
            # slice done: queue its output projection (gated on its norms)
            while early:
                early.popleft()()
            ready = (lambda jj: lambda: norms_done[jj] == HL)(j)
            for qb in range(4 * j, 4 * j + 4):
                filler.extend((ready, c) for c in o_chunk(qb))

        # tail: finish all PVs, then all remaining filler
        drain_pv(len(pvq), min_keep=0)
        while filler:
            filler.popleft()[1]()

    nc.compile()
    return nc


_NC = None


def _get_nc():
    global _NC
    if _NC is None:
        _NC = build_nc()
    return _NC


def _shard_inputs(Query, Key, Value, W_q, W_k, W_v, W_o):
    in_maps = []
    xT = {}
    for b in range(B):
        xT[b] = (
            np.ascontiguousarray(Query[b].T).astype(np.float16),
            np.ascontiguousarray(Key[b].T).astype(np.float16),
            np.ascontiguousarray(Value[b].T).astype(np.float16),
        )
    for b in range(B):
        for hg in range(4):
            r0 = hg * CL
            in_maps.append({
                "xqT": xT[b][0],
                "xkT": xT[b][1],
                "xvT": xT[b][2],
                "wqT": np.ascontiguousarray(W_q[r0:r0 + CL, :].T).astype(np.float16),
                "wkT": np.ascontiguousarray(W_k[r0:r0 + CL, :].T).astype(np.float16),
                "wvT": np.ascontiguousarray(W_v[r0:r0 + CL, :].T).astype(np.float16),
                "woT": np.ascontiguousarray(W_o[:, r0:r0 + CL].T).astype(np.float16),
            })
    return in_maps


def _reference_np(Query, Key, Value, mask, W_q, W_k, W_v, W_o):
    # Fallback for a non-trivial mask (never hit for the spec'd inputs).
    out = np.empty((B, S, D), dtype=np.float32)
    m = np.broadcast_to(mask, (1, 1, S, S))[0, 0]
    for b in range(B):
        Q = (Query[b] @ W_q.T).reshape(S, H, DK).transpose(1, 0, 2)
        K = (Key[b] @ W_k.T).reshape(S, H, DK).transpose(1, 0, 2)
        V = (Value[b] @ W_v.T).reshape(S, H, DK).transpose(1, 0, 2)
        ctx = np.empty((H, S, DK), dtype=np.float32)
        for h in range(H):
            s = (Q[h] @ K[h].T) / np.sqrt(DK)
            s = np.where(m == 0, -1e9, s)
            s -= s.max(axis=-1, keepdims=True)
            e = np.exp(s)
            ctx[h] = (e / e.sum(axis=-1, keepdims=True)) @ V[h]
        out[b] = ctx.transpose(1, 0, 2).reshape(S, D) @ W_o.T
    return out


def kernel(Query, Key, Value, mask, W_q, W_k, W_v, W_o, **_ignored):
    Query = np.asarray(Query, dtype=np.float32)
    Key = np.asarray(Key, dtype=np.float32)
    Value = np.asarray(Value, dtype=np.float32)
    W_q = np.asarray(W_q, dtype=np.float32)
    W_k = np.asarray(W_k, dtype=np.float32)
    W_v = np.asarray(W_v, dtype=np.float32)
    W_o = np.asarray(W_o, dtype=np.float32)

    if not np.all(np.asarray(mask) != 0):
        return _reference_np(Query, Key, Value, np.asarray(mask),
                             W_q, W_k, W_v, W_o)

    nc = _get_nc()
    in_maps = _shard_inputs(Query, Key, Value, W_q, W_k, W_v, W_o)
    res = bass_utils.run_bass_kernel_spmd(nc, in_maps, core_ids=list(range(8)))
    out = np.zeros((B, S, D), dtype=np.float32)
    for b in range(B):
        for hg in range(4):
            out[b] += res.results[b * 4 + hg]["y"].astype(np.float32)
    return out
